# revision 4
# baseline (speedup 1.0000x reference)
"""2-layer GCN (PyG GCNConv semantics) on 8 Trainium2 NeuronCores.

Strategy (dst-sharded, ap_gather, u32-packed bf16 feature pairs):
  - Table: feature-major bf16, packed as u32 pairs (f_2l, f_2l+1) on lane l.
    SBUF slab [128p, 26624]: partition 16b+l = pair l of half(b//4);
    8 bands (b) hold 4 replicas of each src-half.
  - Host: tokens of (core, w, sw, half) round-robin over the half's 4
    bands; padded to 32-multiples with cross-core/band-uniform superchunk
    counts n_sup(w, sw).
  - Device, per layer:
      2 PE matmuls (even/odd feature cols of W) -> psum [16, 512] each;
      DVE pair-evict (x dinv) with stride-2 bf16 writes -> tbl_loc
      [16, 2*NLOCP] bf16 -> AllGather -> slab loads.
      Per window: 1 ap_gather (d=1 f32 container, per-band idx lists,
      window-granular to amortize the ~33us gpsimd wake penalty) ->
      DVE stream_transpose (32x32 u32 blocks) -> mt; bf16 view gives two
      128-token stationaries per block-col: A=[64s,64s+32) (even bands),
      B=[64s+32,64s+64) (odd bands).
      psum[32, 512] += mT.T @ S (one-hot via DVE is_equal);
      window evict: relu(dinv*psum+b) on DVE; layer-2 table build and the
      final Wout matmul are inlined per window.
"""

import sys
import numpy as np

sys.path.insert(0, "/opt/trn_rl_repo")

import ml_dtypes  # noqa: E402

BF16 = ml_dtypes.bfloat16


class Cfg:
    def __init__(self, N, E, F, HID, ACT, NC, WIN, SCOLS):
        self.N, self.E, self.F, self.HID, self.ACT, self.NC = N, E, F, HID, ACT, NC
        self.WIN, self.SCOLS = WIN, SCOLS
        self.NLOC = N // NC
        assert self.NLOC * NC == N
        self.NWIN = -(-self.NLOC // WIN)
        self.NLOCP = self.NWIN * WIN
        assert self.NLOCP % 128 == 0
        self.NSW = WIN // SCOLS
        self.VROWS = NC * self.NLOCP
        self.HALF = self.VROWS // 2
        assert self.HALF < 32768, "int16 gather index overflow"


CFG_FULL = Cfg(N=50000, E=1600000, F=128, HID=32, ACT=64, NC=8, WIN=512, SCOLS=64)


# ----------------------------------------------------------------------------
# Host preprocessing
# ----------------------------------------------------------------------------
def preprocess(x, edge_index, W1, b1, W2, b2, Wout, bout, cfg):
    N, NC, NLOC, NLOCP = cfg.N, cfg.NC, cfg.NLOC, cfg.NLOCP
    NWIN, WIN, NSW, SCOLS = cfg.NWIN, cfg.WIN, cfg.NSW, cfg.SCOLS
    HALF = cfg.HALF

    src = np.asarray(edge_index[0], dtype=np.int64)
    dst = np.asarray(edge_index[1], dtype=np.int64)
    loop = np.arange(N, dtype=np.int64)
    src = np.concatenate([src, loop])
    dst = np.concatenate([dst, loop])
    M = src.shape[0]

    deg = np.bincount(dst, minlength=N).astype(np.float64)
    dinv = np.where(deg > 0, 1.0 / np.sqrt(deg), 0.0).astype(np.float32)

    core = dst // NLOC
    local = dst - core * NLOC
    w = local // WIN
    sw = (local % WIN) // SCOLS
    dcol = (local % SCOLS).astype(np.float32)

    csrc = src // NLOC
    vv = csrc * NLOCP + (src - csrc * NLOC)
    h = vv // HALF
    hrow = (vv - h * HALF).astype(np.int16)

    key = ((core * NWIN + w) * NSW + sw) * 2 + h
    NKEY = NC * NWIN * NSW * 2
    cnt = np.bincount(key, minlength=NKEY)
    order = np.argsort(key, kind="stable")
    ks = key[order]
    gstart = np.concatenate([[0], np.cumsum(cnt)])[:-1]
    pos = np.arange(M) - gstart[ks]

    band_par = pos % 4                   # which of the half's four bands
    bpos = pos // 4

    cnt4 = cnt.reshape(NC, NWIN, NSW, 2)
    nb_hi = -(-cnt4 // 4)                # ceil: max tokens on any band
    n_sup = -(-nb_hi.max(axis=0).max(axis=-1) // 32)    # [NWIN, NSW]
    supbase = np.concatenate([[0], np.cumsum(n_sup.ravel())])
    NSUPT = int(supbase[-1])
    TOKB = 32 * NSUPT                    # per-band padded token count
    supb2 = supbase[:-1].reshape(NWIN, NSW)

    oc = ks // (NWIN * NSW * 2)
    ow = (ks // (NSW * 2)) % NWIN
    osw = (ks // 2) % NSW
    oh = ks % 2
    oband = oh * 4 + band_par            # bands 0-3 half0, 4-7 half1
    sup_glob = supb2[ow, osw] + bpos // 32
    P = 32 * sup_glob + bpos % 32
    jj = bpos % 32

    idx_arr = np.zeros((NC, 8, TOKB), np.int16)
    node_arr = np.full((NC, 128, 2 * NSUPT), -1.0, np.float32)
    idx_arr[oc, oband, P] = hrow[order]
    # stationary a = band parity; partition = 32*(band//2) + jj
    node_arr[oc, 32 * (oband // 2) + jj, 2 * sup_glob + (oband % 2)] = dcol[order]

    nsup_w = n_sup.sum(axis=1)           # [NWIN]
    meta = {
        "n_sup": n_sup,
        "nsup_w": nsup_w,
        "NSUPT": NSUPT,
        "TOKB": TOKB,
        "supb2": supb2,
    }

    x = np.asarray(x, np.float32)
    dinv_pad = np.zeros(NC * NLOCP, np.float32)
    for c in range(NC):
        dinv_pad[c * NLOCP : c * NLOCP + NLOC] = dinv[c * NLOC : (c + 1) * NLOC]

    iota = np.tile(np.arange(SCOLS, dtype=np.float32)[None, :], (128, 1))
    W1 = np.asarray(W1, np.float32)
    W2 = np.asarray(W2, np.float32)
    in_maps = []
    for c in range(NC):
        xc = np.zeros((cfg.F, NLOCP), np.float32)
        xc[:, :NLOC] = x[c * NLOC : (c + 1) * NLOC].T
        dl = dinv_pad[c * NLOCP : (c + 1) * NLOCP]
        idx16 = np.zeros((128, TOKB // 16), np.int16)
        for b in range(8):
            idx16[16 * b : 16 * b + 16] = idx_arr[c, b].reshape(TOKB // 16, 16).T
        in_maps.append(
            {
                "xTb": xc.astype(BF16),
                "W1e": W1[:, 0::2].copy().astype(BF16),
                "W1o": W1[:, 1::2].copy().astype(BF16),
                "W2e": W2[:, 0::2].copy().astype(BF16),
                "W2o": W2[:, 1::2].copy().astype(BF16),
                "Woutb": np.asarray(Wout, np.float32).astype(BF16),
                "b1c": np.asarray(b1, np.float32).reshape(cfg.HID, 1),
                "b2c": np.asarray(b2, np.float32).reshape(cfg.HID, 1),
                "boutc": np.asarray(bout, np.float32).reshape(cfg.ACT, 1),
                "dinvrep": np.tile(dl[None, :], (cfg.HID, 1)).astype(BF16),
                "idx16": np.ascontiguousarray(idx16),
                "node16": np.ascontiguousarray(node_arr[c]).astype(BF16),
                "iota64": iota.astype(BF16),
            }
        )
    return in_maps, meta


# ----------------------------------------------------------------------------
# Device program
# ----------------------------------------------------------------------------
def build(meta, cfg):
    import concourse.mybir as mybir
    import concourse.tile as tile
    from concourse.bacc import Bacc
    from concourse import library_config
    from contextlib import ExitStack

    f32, bf16, i16 = mybir.dt.float32, mybir.dt.bfloat16, mybir.dt.int16
    Alu = mybir.AluOpType
    NWIN, WIN, NSW, SCOLS = cfg.NWIN, cfg.WIN, cfg.NSW, cfg.SCOLS
    HID, F, ACTD = cfg.HID, cfg.F, cfg.ACT
    NLOCP, HALF, NC = cfg.NLOCP, cfg.HALF, cfg.NC
    n_sup, nsup_w = meta["n_sup"], meta["nsup_w"]
    NSUPT, TOKB, supb2 = meta["NSUPT"], meta["TOKB"], meta["supb2"]
    HHID = HID // 2

    nc = Bacc("TRN2", target_bir_lowering=False, debug=False, num_devices=NC)

    xTb = nc.dram_tensor("xTb", [F, NLOCP], bf16, kind="ExternalInput")
    W1e = nc.dram_tensor("W1e", [F, HHID], bf16, kind="ExternalInput")
    W1o = nc.dram_tensor("W1o", [F, HHID], bf16, kind="ExternalInput")
    W2e = nc.dram_tensor("W2e", [HID, HHID], bf16, kind="ExternalInput")
    W2o = nc.dram_tensor("W2o", [HID, HHID], bf16, kind="ExternalInput")
    Woutb = nc.dram_tensor("Woutb", [HID, ACTD], bf16, kind="ExternalInput")
    b1c = nc.dram_tensor("b1c", [HID, 1], f32, kind="ExternalInput")
    b2c = nc.dram_tensor("b2c", [HID, 1], f32, kind="ExternalInput")
    boutc = nc.dram_tensor("boutc", [ACTD, 1], f32, kind="ExternalInput")
    dinvrep = nc.dram_tensor("dinvrep", [HID, NLOCP], bf16, kind="ExternalInput")
    idx16 = nc.dram_tensor("idx16", [128, TOKB // 16], i16, kind="ExternalInput")
    node16 = nc.dram_tensor("node16", [128, 2 * NSUPT], bf16, kind="ExternalInput")
    iota64 = nc.dram_tensor("iota64", [128, SCOLS], bf16, kind="ExternalInput")
    out_fm = nc.dram_tensor("out_fm", [ACTD, NLOCP], f32, kind="ExternalOutput")

    tbl_loc = [nc.dram_tensor(f"tbl_loc{l}", [16, 2 * NLOCP], bf16) for l in (0, 1)]
    tbl_full = [
        nc.dram_tensor(f"tbl_full{l}", [NC * 16, 2 * NLOCP], bf16) for l in (0, 1)
    ]

    with tile.TileContext(nc) as tc:
        nc.gpsimd.load_library(library_config.ap_gather)
        with ExitStack() as ctx:
            consts = ctx.enter_context(tc.tile_pool(name="consts", bufs=1))
            slabp = ctx.enter_context(tc.tile_pool(name="slab", bufs=1))
            xpool = ctx.enter_context(tc.tile_pool(name="xmov", bufs=3))
            gpool = ctx.enter_context(tc.tile_pool(name="gout", bufs=4))
            tpool = ctx.enter_context(tc.tile_pool(name="mT", bufs=2))
            spool = ctx.enter_context(tc.tile_pool(name="sel", bufs=2))
            wkpool = ctx.enter_context(tc.tile_pool(name="work", bufs=2))
            ptbl = ctx.enter_context(tc.tile_pool(name="ptbl", bufs=2, space="PSUM"))
            pagg = ctx.enter_context(tc.tile_pool(name="pagg", bufs=2, space="PSUM"))
            pout = ctx.enter_context(tc.tile_pool(name="pout", bufs=2, space="PSUM"))

            w1e_t = consts.tile([F, HHID], bf16)
            nc.sync.dma_start(w1e_t[:], W1e[:])
            w1o_t = consts.tile([F, HHID], bf16)
            nc.sync.dma_start(w1o_t[:], W1o[:])
            w2e_t = consts.tile([HID, HHID], bf16)
            nc.sync.dma_start(w2e_t[:], W2e[:])
            w2o_t = consts.tile([HID, HHID], bf16)
            nc.sync.dma_start(w2o_t[:], W2o[:])
            wout_t = consts.tile([HID, ACTD], bf16)
            nc.sync.dma_start(wout_t[:], Woutb[:])
            b1_t = consts.tile([HID, 1], f32)
            nc.sync.dma_start(b1_t[:], b1c[:])
            b2_t = consts.tile([HID, 1], f32)
            nc.sync.dma_start(b2_t[:], b2c[:])
            bout_t = consts.tile([ACTD, 1], f32)
            nc.sync.dma_start(bout_t[:], boutc[:])
            dinv_t = consts.tile([HID, NLOCP], bf16)
            nc.sync.dma_start(dinv_t[:], dinvrep[:])
            idx_t = consts.tile([128, TOKB // 16], i16)
            nc.sync.dma_start(idx_t[:], idx16[:])
            node_t = consts.tile([128, 2 * NSUPT], bf16)
            nc.sync.dma_start(node_t[:], node16[:])
            iota_t = consts.tile([128, SCOLS], bf16)
            nc.sync.dma_start(iota_t[:], iota64[:])

            slab_t = slabp.tile([128, HALF], f32)

            def pair_evict(l, pse, pso, cols, name):
                """tbl_loc[l] pair-interleaved bf16 write: x dinv."""
                ev = wkpool.tile([16, 2 * WIN], bf16, tag="tblev", name=name)
                ev3 = ev[:].rearrange("p (j k) -> p j k", k=2)
                nc.vector.tensor_tensor(
                    ev3[:, :, 0], pse[:], dinv_t[0:16, cols], op=Alu.mult
                )
                nc.vector.tensor_tensor(
                    ev3[:, :, 1], pso[:], dinv_t[0:16, cols], op=Alu.mult
                )
                w0 = cols.start
                nc.sync.dma_start(
                    tbl_loc[l][:, 2 * w0 : 2 * w0 + 2 * WIN], ev[:]
                )

            def build_table1():
                for b in range(NWIN):
                    cols = slice(b * WIN, (b + 1) * WIN)
                    xm = xpool.tile([F, WIN], bf16, tag="xm")
                    nc.sync.dma_start(xm[:], xTb[:, cols])
                    pse = ptbl.tile([HHID, WIN], f32, tag="ptbe")
                    nc.tensor.matmul(pse[:], w1e_t[:], xm[:], start=True, stop=True)
                    pso = ptbl.tile([HHID, WIN], f32, tag="ptbo")
                    nc.tensor.matmul(pso[:], w1o_t[:], xm[:], start=True, stop=True)
                    pair_evict(0, pse, pso, cols, f"ev0_{b}")

            def gather_load(l):
                nc.gpsimd.collective_compute(
                    "AllGather",
                    Alu.bypass,
                    replica_groups=[list(range(NC))],
                    ins=[tbl_loc[l].ap().opt()],
                    outs=[tbl_full[l].ap().opt()],
                )
                for b in range(8):
                    half = b // 4
                    rows = slice(half * 64, half * 64 + 64)
                    src_ap = tbl_full[l][rows, :].rearrange("(c f) j -> f c j", c=4)
                    dst_ap = (
                        slab_t[16 * b : 16 * b + 16, :]
                        .bitcast(bf16)
                        .rearrange("p (c j) -> p c j", c=4)
                    )
                    nc.sync.dma_start(dst_ap, src_ap)
                # collapse the gather's dependency fan-in (8 DMA sems, each a
                # ~33us gpsimd park) into one DVE sem via an in-place pass
                nc.any.tensor_copy(out=slab_t[:], in_=slab_t[:])

            def aggregate(l, bias_t):
                staged = {}

                def prep(w):
                    nsw_ = int(nsup_w[w])
                    supw0 = int(supb2[w, 0])
                    nidx = 32 * nsw_
                    go = gpool.tile([128, nidx], f32, tag="go", name=f"go{l}_{w}")
                    nc.gpsimd.ap_gather(
                        go[:].unsqueeze(-1),
                        slab_t[:].unsqueeze(-1),
                        idx_t[:, 2 * supw0 : 2 * supw0 + nidx // 16],
                        channels=128,
                        num_elems=HALF,
                        d=1,
                        num_idxs=nidx,
                    )
                    mt = tpool.tile([128, nidx], f32, tag="mt", name=f"mt{l}_{w}")
                    nc.vector.transpose(mt[:], go[:])
                    staged[w] = mt

                def process(w):
                    mt = staged.pop(w)
                    mtb = mt[:].bitcast(bf16)
                    supw0 = int(supb2[w, 0])
                    ps = pagg.tile([HID, WIN], f32, tag="pagg", name=f"ps{l}_{w}")
                    total_mm = 2 * int(nsup_w[w])
                    done = 0
                    s_off = 0
                    for qw in range(4):
                        nq = int(n_sup[w, 2 * qw] + n_sup[w, 2 * qw + 1])
                        if nq == 0:
                            continue
                        sq0 = supb2[w, 2 * qw]
                        st = spool.tile(
                            [128, 2 * nq, SCOLS], bf16, tag="st", name=f"st{l}_{w}_{qw}"
                        )
                        nc.vector.tensor_tensor(
                            st[:],
                            node_t[
                                :, 2 * sq0 : 2 * sq0 + 2 * nq, None
                            ].to_broadcast([128, 2 * nq, SCOLS]),
                            iota_t[:, None, :].to_broadcast([128, 2 * nq, SCOLS]),
                            op=Alu.is_equal,
                        )
                        sq_off = 0
                        for sw in (2 * qw, 2 * qw + 1):
                            for k in range(int(n_sup[w, sw])):
                                for a in (0, 1):
                                    nc.tensor.matmul(
                                        ps[:, sw * SCOLS : (sw + 1) * SCOLS],
                                        mtb[:, 64 * s_off + 32 * a :
                                            64 * s_off + 32 * a + 32],
                                        st[:, 2 * sq_off + a, :],
                                        start=done == 0,
                                        stop=done == total_mm - 1,
                                    )
                                    done += 1
                                s_off += 1
                                sq_off += 1
                    # window evict
                    cols = slice(w * WIN, (w + 1) * WIN)
                    tmp = wkpool.tile([HID, WIN], f32, tag="aggev", name=f"tm{l}_{w}")
                    nc.vector.tensor_tensor(
                        tmp[:], ps[:], dinv_t[:, cols], op=Alu.mult
                    )
                    ow_ = wkpool.tile([HID, WIN], bf16, tag="ow", name=f"ow{l}_{w}")
                    nc.vector.tensor_scalar(
                        ow_[:], tmp[:], bias_t[:], 0.0, op0=Alu.add, op1=Alu.max
                    )
                    if l == 0:
                        # inline layer-2 table build for this window
                        pse = ptbl.tile([HHID, WIN], f32, tag="ptbe", name=f"p2e{w}")
                        nc.tensor.matmul(
                            pse[:], w2e_t[:], ow_[:], start=True, stop=True
                        )
                        pso = ptbl.tile([HHID, WIN], f32, tag="ptbo", name=f"p2o{w}")
                        nc.tensor.matmul(
                            pso[:], w2o_t[:], ow_[:], start=True, stop=True
                        )
                        pair_evict(1, pse, pso, cols, f"ev1_{w}")
                    else:
                        pso_f = pout.tile([ACTD, WIN], f32, tag="pl", name=f"pf{w}")
                        nc.tensor.matmul(
                            pso_f[:], wout_t[:], ow_[:], start=True, stop=True
                        )
                        lsb = wkpool.tile([ACTD, WIN], f32, tag="lsb", name=f"lb{w}")
                        nc.vector.tensor_scalar_add(lsb[:], pso_f[:], bout_t[:])
                        nc.sync.dma_start(out_fm[:, cols], lsb[:])

                prep(0)
                for w in range(NWIN):
                    if w + 1 < NWIN:
                        prep(w + 1)
                    process(w)

            build_table1()
            gather_load(0)
            aggregate(0, b1_t)
            gather_load(1)
            aggregate(1, b2_t)

    nc.compile()
    return nc


# ----------------------------------------------------------------------------
# Entry point
# ----------------------------------------------------------------------------
_CACHE = {}


def run(x, edge_index, W1, b1, W2, b2, Wout, bout, cfg, trace=False):
    from concourse import bass_utils

    in_maps, meta = preprocess(x, edge_index, W1, b1, W2, b2, Wout, bout, cfg)
    key = (cfg.N, cfg.E, meta["NSUPT"], tuple(meta["n_sup"].ravel().tolist()))
    if key not in _CACHE:
        _CACHE[key] = build(meta, cfg)
    nc = _CACHE[key]
    res = bass_utils.run_bass_kernel_spmd(
        nc, in_maps, core_ids=list(range(cfg.NC)), trace=trace
    )
    out = np.empty((cfg.N, cfg.ACT), np.float32)
    for c in range(cfg.NC):
        out[c * cfg.NLOC : (c + 1) * cfg.NLOC] = (
            res.results[c]["out_fm"][:, : cfg.NLOC].T
        )
    return out, res


def kernel(x, edge_index, W1, b1, W2, b2, Wout, bout):
    out, _ = run(x, edge_index, W1, b1, W2, b2, Wout, bout, CFG_FULL)
    return out


# revision 7
# speedup vs baseline: 1.0066x; 1.0066x over previous
"""2-layer GCN (PyG GCNConv semantics) on 8 Trainium2 NeuronCores.

Strategy (dst-sharded, ap_gather, u32-packed bf16 feature pairs):
  - Table: feature-major bf16, packed as u32 pairs (f_2l, f_2l+1) on lane l.
    SBUF slab [128p, 26624]: partition 16b+l = pair l of half(b//4);
    8 bands (b) hold 4 replicas of each src-half.
  - Host: tokens of (core, w, sw, half) round-robin over the half's 4
    bands; padded to 32-multiples with cross-core/band-uniform superchunk
    counts n_sup(w, sw).
  - Device, per layer:
      2 PE matmuls (even/odd feature cols of W) -> psum [16, 512] each;
      DVE pair-evict (x dinv) with stride-2 bf16 writes -> tbl_loc
      [16, 2*NLOCP] bf16 -> AllGather -> slab loads.
      Per window: 1 ap_gather (d=1 f32 container, per-band idx lists,
      window-granular to amortize the ~33us gpsimd wake penalty) ->
      DVE stream_transpose (32x32 u32 blocks) -> mt; bf16 view gives two
      128-token stationaries per block-col: A=[64s,64s+32) (even bands),
      B=[64s+32,64s+64) (odd bands).
      psum[32, 512] += mT.T @ S (one-hot via DVE is_equal);
      window evict: relu(dinv*psum+b) on DVE; layer-2 table build and the
      final Wout matmul are inlined per window.
"""

import sys
import numpy as np

sys.path.insert(0, "/opt/trn_rl_repo")

import ml_dtypes  # noqa: E402

BF16 = ml_dtypes.bfloat16


class Cfg:
    def __init__(self, N, E, F, HID, ACT, NC, WIN, SCOLS):
        self.N, self.E, self.F, self.HID, self.ACT, self.NC = N, E, F, HID, ACT, NC
        self.WIN, self.SCOLS = WIN, SCOLS
        self.NLOC = N // NC
        assert self.NLOC * NC == N
        self.NWIN = -(-self.NLOC // WIN)
        self.NLOCP = self.NWIN * WIN
        assert self.NLOCP % 128 == 0
        self.NSW = WIN // SCOLS
        self.VROWS = NC * self.NLOCP
        self.HALF = self.VROWS // 2
        assert self.HALF < 32768, "int16 gather index overflow"


CFG_FULL = Cfg(N=50000, E=1600000, F=128, HID=32, ACT=64, NC=8, WIN=512, SCOLS=64)


# ----------------------------------------------------------------------------
# Host preprocessing
# ----------------------------------------------------------------------------
def preprocess(x, edge_index, W1, b1, W2, b2, Wout, bout, cfg):
    N, NC, NLOC, NLOCP = cfg.N, cfg.NC, cfg.NLOC, cfg.NLOCP
    NWIN, WIN, NSW, SCOLS = cfg.NWIN, cfg.WIN, cfg.NSW, cfg.SCOLS
    HALF = cfg.HALF

    src = np.asarray(edge_index[0], dtype=np.int64)
    dst = np.asarray(edge_index[1], dtype=np.int64)
    loop = np.arange(N, dtype=np.int64)
    src = np.concatenate([src, loop])
    dst = np.concatenate([dst, loop])
    M = src.shape[0]

    deg = np.bincount(dst, minlength=N).astype(np.float64)
    dinv = np.where(deg > 0, 1.0 / np.sqrt(deg), 0.0).astype(np.float32)

    core = dst // NLOC
    local = dst - core * NLOC
    w = local // WIN
    sw = (local % WIN) // SCOLS
    dcol = (local % SCOLS).astype(np.float32)

    csrc = src // NLOC
    vv = csrc * NLOCP + (src - csrc * NLOC)
    h = vv // HALF
    hrow = (vv - h * HALF).astype(np.int16)

    key = ((core * NWIN + w) * NSW + sw) * 2 + h
    NKEY = NC * NWIN * NSW * 2
    cnt = np.bincount(key, minlength=NKEY)
    order = np.argsort(key, kind="stable")
    ks = key[order]
    gstart = np.concatenate([[0], np.cumsum(cnt)])[:-1]
    pos = np.arange(M) - gstart[ks]

    band_par = pos % 4                   # which of the half's four bands
    bpos = pos // 4

    cnt4 = cnt.reshape(NC, NWIN, NSW, 2)
    nb_hi = -(-cnt4 // 4)                # ceil: max tokens on any band
    n_sup = -(-nb_hi.max(axis=0).max(axis=-1) // 32)    # [NWIN, NSW]
    supbase = np.concatenate([[0], np.cumsum(n_sup.ravel())])
    NSUPT = int(supbase[-1])
    TOKB = 32 * NSUPT                    # per-band padded token count
    supb2 = supbase[:-1].reshape(NWIN, NSW)

    oc = ks // (NWIN * NSW * 2)
    ow = (ks // (NSW * 2)) % NWIN
    osw = (ks // 2) % NSW
    oh = ks % 2
    oband = oh * 4 + band_par            # bands 0-3 half0, 4-7 half1
    sup_glob = supb2[ow, osw] + bpos // 32
    P = 32 * sup_glob + bpos % 32
    jj = bpos % 32

    idx_arr = np.zeros((NC, 8, TOKB), np.int16)
    node_arr = np.full((NC, 128, 2 * NSUPT), -1.0, np.float32)
    idx_arr[oc, oband, P] = hrow[order]
    # stationary a = band parity; partition = 32*(band//2) + jj
    node_arr[oc, 32 * (oband // 2) + jj, 2 * sup_glob + (oband % 2)] = dcol[order]

    nsup_w = n_sup.sum(axis=1)           # [NWIN]
    meta = {
        "n_sup": n_sup,
        "nsup_w": nsup_w,
        "NSUPT": NSUPT,
        "TOKB": TOKB,
        "supb2": supb2,
    }

    x = np.asarray(x, np.float32)
    dinv_pad = np.zeros(NC * NLOCP, np.float32)
    for c in range(NC):
        dinv_pad[c * NLOCP : c * NLOCP + NLOC] = dinv[c * NLOC : (c + 1) * NLOC]

    iota = np.tile(np.arange(SCOLS, dtype=np.float32)[None, :], (128, 1))
    W1 = np.asarray(W1, np.float32)
    W2 = np.asarray(W2, np.float32)
    in_maps = []
    for c in range(NC):
        xc = np.zeros((cfg.F, NLOCP), np.float32)
        xc[:, :NLOC] = x[c * NLOC : (c + 1) * NLOC].T
        dl = dinv_pad[c * NLOCP : (c + 1) * NLOCP]
        idx16 = np.zeros((128, TOKB // 16), np.int16)
        for b in range(8):
            idx16[16 * b : 16 * b + 16] = idx_arr[c, b].reshape(TOKB // 16, 16).T
        in_maps.append(
            {
                "xTb": xc.astype(BF16),
                "W1e": W1[:, 0::2].copy().astype(BF16),
                "W1o": W1[:, 1::2].copy().astype(BF16),
                "W2e": W2[:, 0::2].copy().astype(BF16),
                "W2o": W2[:, 1::2].copy().astype(BF16),
                "Woutb": np.asarray(Wout, np.float32).astype(BF16),
                "b1c": np.asarray(b1, np.float32).reshape(cfg.HID, 1),
                "b2c": np.asarray(b2, np.float32).reshape(cfg.HID, 1),
                "boutc": np.asarray(bout, np.float32).reshape(cfg.ACT, 1),
                "dinvrep": np.tile(dl[None, :], (cfg.HID, 1)).astype(BF16),
                "idx16": np.ascontiguousarray(idx16),
                "dzero": np.zeros((128, 128), np.int16),
                "node16": np.ascontiguousarray(node_arr[c]).astype(BF16),
                "iota64": iota.astype(BF16),
            }
        )
    return in_maps, meta


# ----------------------------------------------------------------------------
# Device program
# ----------------------------------------------------------------------------
def build(meta, cfg):
    import concourse.mybir as mybir
    import concourse.tile as tile
    from concourse.bacc import Bacc
    from concourse import library_config
    from contextlib import ExitStack

    f32, bf16, i16 = mybir.dt.float32, mybir.dt.bfloat16, mybir.dt.int16
    Alu = mybir.AluOpType
    NWIN, WIN, NSW, SCOLS = cfg.NWIN, cfg.WIN, cfg.NSW, cfg.SCOLS
    HID, F, ACTD = cfg.HID, cfg.F, cfg.ACT
    NLOCP, HALF, NC = cfg.NLOCP, cfg.HALF, cfg.NC
    n_sup, nsup_w = meta["n_sup"], meta["nsup_w"]
    NSUPT, TOKB, supb2 = meta["NSUPT"], meta["TOKB"], meta["supb2"]
    HHID = HID // 2

    nc = Bacc("TRN2", target_bir_lowering=False, debug=False, num_devices=NC)

    xTb = nc.dram_tensor("xTb", [F, NLOCP], bf16, kind="ExternalInput")
    W1e = nc.dram_tensor("W1e", [F, HHID], bf16, kind="ExternalInput")
    W1o = nc.dram_tensor("W1o", [F, HHID], bf16, kind="ExternalInput")
    W2e = nc.dram_tensor("W2e", [HID, HHID], bf16, kind="ExternalInput")
    W2o = nc.dram_tensor("W2o", [HID, HHID], bf16, kind="ExternalInput")
    Woutb = nc.dram_tensor("Woutb", [HID, ACTD], bf16, kind="ExternalInput")
    b1c = nc.dram_tensor("b1c", [HID, 1], f32, kind="ExternalInput")
    b2c = nc.dram_tensor("b2c", [HID, 1], f32, kind="ExternalInput")
    boutc = nc.dram_tensor("boutc", [ACTD, 1], f32, kind="ExternalInput")
    dinvrep = nc.dram_tensor("dinvrep", [HID, NLOCP], bf16, kind="ExternalInput")
    idx16 = nc.dram_tensor("idx16", [128, TOKB // 16], i16, kind="ExternalInput")
    dzero = nc.dram_tensor("dzero", [128, 128], i16, kind="ExternalInput")
    node16 = nc.dram_tensor("node16", [128, 2 * NSUPT], bf16, kind="ExternalInput")
    iota64 = nc.dram_tensor("iota64", [128, SCOLS], bf16, kind="ExternalInput")
    out_fm = nc.dram_tensor("out_fm", [ACTD, NLOCP], f32, kind="ExternalOutput")

    tbl_loc = [nc.dram_tensor(f"tbl_loc{l}", [16, 2 * NLOCP], bf16) for l in (0, 1)]
    tbl_full = [
        nc.dram_tensor(f"tbl_full{l}", [NC * 16, 2 * NLOCP], bf16) for l in (0, 1)
    ]

    with tile.TileContext(nc) as tc:
        nc.gpsimd.load_library(library_config.ap_gather)
        with ExitStack() as ctx:
            consts = ctx.enter_context(tc.tile_pool(name="consts", bufs=1))
            slabp = ctx.enter_context(tc.tile_pool(name="slab", bufs=1))
            xpool = ctx.enter_context(tc.tile_pool(name="xmov", bufs=3))
            gpool = ctx.enter_context(tc.tile_pool(name="gout", bufs=3))
            tpool = ctx.enter_context(tc.tile_pool(name="mT", bufs=2))
            spool = ctx.enter_context(tc.tile_pool(name="sel", bufs=2))
            wkpool = ctx.enter_context(tc.tile_pool(name="work", bufs=2))
            ptbl = ctx.enter_context(tc.tile_pool(name="ptbl", bufs=2, space="PSUM"))
            pagg = ctx.enter_context(tc.tile_pool(name="pagg", bufs=2, space="PSUM"))
            pout = ctx.enter_context(tc.tile_pool(name="pout", bufs=2, space="PSUM"))

            w1e_t = consts.tile([F, HHID], bf16)
            nc.sync.dma_start(w1e_t[:], W1e[:])
            w1o_t = consts.tile([F, HHID], bf16)
            nc.sync.dma_start(w1o_t[:], W1o[:])
            w2e_t = consts.tile([HID, HHID], bf16)
            nc.sync.dma_start(w2e_t[:], W2e[:])
            w2o_t = consts.tile([HID, HHID], bf16)
            nc.sync.dma_start(w2o_t[:], W2o[:])
            wout_t = consts.tile([HID, ACTD], bf16)
            nc.sync.dma_start(wout_t[:], Woutb[:])
            b1_t = consts.tile([HID, 1], f32)
            nc.sync.dma_start(b1_t[:], b1c[:])
            b2_t = consts.tile([HID, 1], f32)
            nc.sync.dma_start(b2_t[:], b2c[:])
            bout_t = consts.tile([ACTD, 1], f32)
            nc.sync.dma_start(bout_t[:], boutc[:])
            dinv_t = consts.tile([HID, NLOCP], bf16)
            nc.sync.dma_start(dinv_t[:], dinvrep[:])
            idx_t = consts.tile([128, TOKB // 16], i16)
            nc.sync.dma_start(idx_t[:], idx16[:])
            dz_t = consts.tile([128, 128], i16)
            nc.sync.dma_start(dz_t[:], dzero[:])
            dscr = consts.tile([128, 4096], i16)
            node_t = consts.tile([128, 2 * NSUPT], bf16)
            nc.sync.dma_start(node_t[:], node16[:])
            iota_t = consts.tile([128, SCOLS], bf16)
            nc.sync.dma_start(iota_t[:], iota64[:])

            slab_t = slabp.tile([128, HALF], f32)

            def pair_evict(l, pse, pso, cols, name):
                """tbl_loc[l] pair-interleaved bf16 write: x dinv."""
                ev = wkpool.tile([16, 2 * WIN], bf16, tag="tblev", name=name)
                ev3 = ev[:].rearrange("p (j k) -> p j k", k=2)
                nc.vector.tensor_tensor(
                    ev3[:, :, 0], pse[:], dinv_t[0:16, cols], op=Alu.mult
                )
                nc.vector.tensor_tensor(
                    ev3[:, :, 1], pso[:], dinv_t[0:16, cols], op=Alu.mult
                )
                w0 = cols.start
                nc.sync.dma_start(
                    tbl_loc[l][:, 2 * w0 : 2 * w0 + 2 * WIN], ev[:]
                )

            def build_table1():
                for b in range(NWIN):
                    cols = slice(b * WIN, (b + 1) * WIN)
                    xm = xpool.tile([F, WIN], bf16, tag="xm")
                    nc.sync.dma_start(xm[:], xTb[:, cols])
                    pse = ptbl.tile([HHID, WIN], f32, tag="ptbe")
                    nc.tensor.matmul(pse[:], w1e_t[:], xm[:], start=True, stop=True)
                    pso = ptbl.tile([HHID, WIN], f32, tag="ptbo")
                    nc.tensor.matmul(pso[:], w1o_t[:], xm[:], start=True, stop=True)
                    pair_evict(0, pse, pso, cols, f"ev0_{b}")

            def gather_load(l):
                nc.gpsimd.collective_compute(
                    "AllGather",
                    Alu.bypass,
                    replica_groups=[list(range(NC))],
                    ins=[tbl_loc[l].ap().opt()],
                    outs=[tbl_full[l].ap().opt()],
                )
                for b in range(8):
                    half = b // 4
                    for ci in range(4):
                        c = half * 4 + ci
                        nc.sync.dma_start(
                            slab_t[16 * b : 16 * b + 16,
                                   ci * NLOCP : (ci + 1) * NLOCP].bitcast(bf16),
                            tbl_full[l][c * 16 : (c + 1) * 16, :],
                        )

            def aggregate(l, bias_t):
                # keep the Q7 busy (~64us) while the AllGather + slab loads
                # land, so the first real gather's waits are satisfied at
                # check time (else each unsatisfied gpsimd wait parks ~33us)
                nc.gpsimd.ap_gather(
                    dscr[:].rearrange("p (n d) -> p n d", d=2),
                    dz_t[:].rearrange("p (n d) -> p n d", d=2),
                    dz_t[:],
                    channels=128,
                    num_elems=64,
                    d=2,
                    num_idxs=2048,
                )
                staged = {}

                def prep(w):
                    nsw_ = int(nsup_w[w])
                    supw0 = int(supb2[w, 0])
                    nidx = 32 * nsw_
                    go = gpool.tile([128, nidx], f32, tag="go", name=f"go{l}_{w}")
                    nc.gpsimd.ap_gather(
                        go[:].unsqueeze(-1),
                        slab_t[:].unsqueeze(-1),
                        idx_t[:, 2 * supw0 : 2 * supw0 + nidx // 16],
                        channels=128,
                        num_elems=HALF,
                        d=1,
                        num_idxs=nidx,
                    )
                    mt = tpool.tile([128, nidx], f32, tag="mt", name=f"mt{l}_{w}")
                    nc.vector.transpose(mt[:], go[:])
                    staged[w] = mt

                def process(w):
                    mt = staged.pop(w)
                    mtb = mt[:].bitcast(bf16)
                    supw0 = int(supb2[w, 0])
                    ps = pagg.tile([HID, WIN], f32, tag="pagg", name=f"ps{l}_{w}")
                    total_mm = 2 * int(nsup_w[w])
                    done = 0
                    s_off = 0
                    for qw in range(4):
                        nq = int(n_sup[w, 2 * qw] + n_sup[w, 2 * qw + 1])
                        if nq == 0:
                            continue
                        sq0 = supb2[w, 2 * qw]
                        st = spool.tile(
                            [128, 2 * nq, SCOLS], bf16, tag="st", name=f"st{l}_{w}_{qw}"
                        )
                        nc.vector.tensor_tensor(
                            st[:],
                            node_t[
                                :, 2 * sq0 : 2 * sq0 + 2 * nq, None
                            ].to_broadcast([128, 2 * nq, SCOLS]),
                            iota_t[:, None, :].to_broadcast([128, 2 * nq, SCOLS]),
                            op=Alu.is_equal,
                        )
                        sq_off = 0
                        for sw in (2 * qw, 2 * qw + 1):
                            for k in range(int(n_sup[w, sw])):
                                for a in (0, 1):
                                    nc.tensor.matmul(
                                        ps[:, sw * SCOLS : (sw + 1) * SCOLS],
                                        mtb[:, 64 * s_off + 32 * a :
                                            64 * s_off + 32 * a + 32],
                                        st[:, 2 * sq_off + a, :],
                                        start=done == 0,
                                        stop=done == total_mm - 1,
                                    )
                                    done += 1
                                s_off += 1
                                sq_off += 1
                    # window evict
                    cols = slice(w * WIN, (w + 1) * WIN)
                    tmp = wkpool.tile([HID, WIN], f32, tag="aggev", name=f"tm{l}_{w}")
                    nc.vector.tensor_tensor(
                        tmp[:], ps[:], dinv_t[:, cols], op=Alu.mult
                    )
                    ow_ = wkpool.tile([HID, WIN], bf16, tag="ow", name=f"ow{l}_{w}")
                    nc.vector.tensor_scalar(
                        ow_[:], tmp[:], bias_t[:], 0.0, op0=Alu.add, op1=Alu.max
                    )
                    if l == 0:
                        # inline layer-2 table build for this window
                        pse = ptbl.tile([HHID, WIN], f32, tag="ptbe", name=f"p2e{w}")
                        nc.tensor.matmul(
                            pse[:], w2e_t[:], ow_[:], start=True, stop=True
                        )
                        pso = ptbl.tile([HHID, WIN], f32, tag="ptbo", name=f"p2o{w}")
                        nc.tensor.matmul(
                            pso[:], w2o_t[:], ow_[:], start=True, stop=True
                        )
                        pair_evict(1, pse, pso, cols, f"ev1_{w}")
                    else:
                        pso_f = pout.tile([ACTD, WIN], f32, tag="pl", name=f"pf{w}")
                        nc.tensor.matmul(
                            pso_f[:], wout_t[:], ow_[:], start=True, stop=True
                        )
                        lsb = wkpool.tile([ACTD, WIN], f32, tag="lsb", name=f"lb{w}")
                        nc.vector.tensor_scalar_add(lsb[:], pso_f[:], bout_t[:])
                        nc.sync.dma_start(out_fm[:, cols], lsb[:])

                prep(0)
                for w in range(NWIN):
                    if w + 1 < NWIN:
                        prep(w + 1)
                    process(w)

            build_table1()
            gather_load(0)
            aggregate(0, b1_t)
            gather_load(1)
            aggregate(1, b2_t)

    nc.compile()
    return nc


# ----------------------------------------------------------------------------
# Entry point
# ----------------------------------------------------------------------------
_CACHE = {}


def run(x, edge_index, W1, b1, W2, b2, Wout, bout, cfg, trace=False):
    from concourse import bass_utils

    in_maps, meta = preprocess(x, edge_index, W1, b1, W2, b2, Wout, bout, cfg)
    key = (cfg.N, cfg.E, meta["NSUPT"], tuple(meta["n_sup"].ravel().tolist()))
    if key not in _CACHE:
        _CACHE[key] = build(meta, cfg)
    nc = _CACHE[key]
    res = bass_utils.run_bass_kernel_spmd(
        nc, in_maps, core_ids=list(range(cfg.NC)), trace=trace
    )
    out = np.empty((cfg.N, cfg.ACT), np.float32)
    for c in range(cfg.NC):
        out[c * cfg.NLOC : (c + 1) * cfg.NLOC] = (
            res.results[c]["out_fm"][:, : cfg.NLOC].T
        )
    return out, res


def kernel(x, edge_index, W1, b1, W2, b2, Wout, bout):
    out, _ = run(x, edge_index, W1, b1, W2, b2, Wout, bout, CFG_FULL)
    return out
